# revision 6
# baseline (speedup 1.0000x reference)
"""MinkowskiFlow coarse-flow kernel for 8 Trainium2 NeuronCores (Bass/Tile).

Math (per batch b):
    fs = normalize(feat_s); ft = normalize(feat_t)
    C[n,m]   = 2 - 2 <fs_n, ft_m>
    K[n,m]   = exp(-C/(exp(eps)+0.03)) * (||coor_s_n - coor_t_m||^2 < 100)
    out[n,:] = (K @ coor_t) / (sum_m K + 1e-8) - coor_s

Sharding: batch b -> 4 cores each (data parallel over B=2), N split into 4
row blocks of 1024 (row-wise; each row's normalization is independent).

Per-core layout (all "transposed": target index m on SBUF partitions):
    S^T[m,n]  via PE matmul, bf16 hi/lo split of normalized features
              (2 accumulating passes; drops only the lo*lo term ~1e-6)
    dist^T    via PE fp32 matmul on CENTERED coords (kills the
              a2+b2-2ab cancellation; knife-edge radius pairs carry up
              to 31% of a row's softmax weight, so this must be fp32)
              R'[m,n] = -2*ctc_m . csc_n + |csc_n|^2 ;  mask = R' < 100-|ctc_m|^2
    K^T       = exp((2 S^T - 2)/tau) * mask   (one ACT op + one fused DVE op)
    agg       = [coor_t | 1]^T @ K^T -> [4, n] PSUM accumulated over all
              m tiles: rows 0-2 = sum K*coor_t, row 3 = row_sum.
Final per n-tile: PE-transpose agg slice, out = acc*recip(rs+1e-8) - coor_s.
"""
import numpy as np
from contextlib import ExitStack

import concourse.bass as bass
import concourse.bacc as bacc
import concourse.tile as tile
import concourse.mybir as mybir
from concourse import masks
from concourse.bass_utils import run_bass_kernel_spmd

F32 = mybir.dt.float32
F32R = mybir.dt.float32r
BF16 = mybir.dt.bfloat16
AF = mybir.ActivationFunctionType
ALU = mybir.AluOpType

B, N, M, D = 2, 4096, 4096, 64
N_CORES = 8
CORES_PER_BATCH = N_CORES // B      # 4
NS = N // CORES_PER_BATCH           # 1024 source rows per core
P = 128
MT = M // P                         # 32 target tiles
NT = NS // P                        # 8 source tiles per core
CHUNK = 512
NCHUNK = NS // CHUNK                # 2
CENTER = 20.0
TAU_OFFSET = 0.03
RADIUS_SQ = 100.0

AGG_DT = F32      # dtype of K tiles / ct_aug for the aggregation matmul
SPLIT_LO_MIXED = True  # lo = f32 - bf16(hi) in one mixed-dtype DVE op


def build_kernel(tau: float):
    nc = bacc.Bacc("TRN2", target_bir_lowering=False, debug=False,
                   num_devices=N_CORES)
    fs_d = nc.dram_tensor("fs", [NS, D], F32, kind="ExternalInput").ap()
    ft_d = nc.dram_tensor("ft", [M, D], F32, kind="ExternalInput").ap()
    cs_d = nc.dram_tensor("cs", [NS, 3], F32, kind="ExternalInput").ap()
    ct_d = nc.dram_tensor("ct", [M, 3], F32, kind="ExternalInput").ap()
    out_d = nc.dram_tensor("out", [NS, 3], F32, kind="ExternalOutput").ap()

    scale = float(2.0 / tau)

    with tile.TileContext(nc) as tc, ExitStack() as ctx:
        pers = ctx.enter_context(tc.tile_pool(name="pers", bufs=1))
        scr = ctx.enter_context(tc.tile_pool(name="scr", bufs=3))
        sbE = ctx.enter_context(tc.tile_pool(name="sbE", bufs=3))
        sbK = ctx.enter_context(tc.tile_pool(name="sbK", bufs=3))
        fin = ctx.enter_context(tc.tile_pool(name="fin", bufs=2))
        psA = ctx.enter_context(tc.tile_pool(name="psA", bufs=2, space="PSUM"))
        psB = ctx.enter_context(tc.tile_pool(name="psB", bufs=2, space="PSUM"))
        psG = ctx.enter_context(tc.tile_pool(name="psG", bufs=1, space="PSUM"))
        psS = ctx.enter_context(tc.tile_pool(name="psS", bufs=2, space="PSUM"))

        # ---------------- persistent tensors ----------------
        ftT = pers.tile([P, M], BF16)          # rows 0:64 ft_hi^T, 64:128 ft_lo^T
        rhsA = pers.tile([P, NS], BF16)        # fs_hi^T duplicated on both halves
        rhsB = pers.tile([64, NS], BF16)       # fs_lo^T
        ctT = pers.tile([4, M], F32)           # rows 0:3 -2*(ct-20)^T, row 3 ones
        rhsC = pers.tile([4, NS], F32)         # rows 0:3 (cs-20)^T, row 3 |cs-20|^2
        thr = pers.tile([P, MT], F32)          # 100 - |ct-20|^2, column per m-tile
        ct_aug = pers.tile([P, 4 * MT], AGG_DT)  # [ct_x ct_y ct_z 1] per m-tile
        ident = pers.tile([P, P], F32)
        identb = pers.tile([P, P], BF16)
        biasT = pers.tile([P, 1], F32)

        ft_all = pers.tile([P, MT * D], F32)
        fs_all = pers.tile([P, NT * D], F32)
        ct_all = pers.tile([P, MT * 3], F32)
        cs_all = pers.tile([P, NT * 3], F32)
        s2t = pers.tile([P, MT], F32)
        s2s = pers.tile([P, NT], F32)
        ct2c = pers.tile([P, MT], F32)
        cs2c = pers.tile([P, NT], F32)
        fhi_t = pers.tile([P, MT * D], BF16)
        flo_t = pers.tile([P, MT * D], BF16)
        fhi_s = pers.tile([P, NT * D], BF16)
        flo_s = pers.tile([P, NT * D], BF16)

        masks.make_identity(nc, ident[:])
        masks.make_identity(nc, identb[:])
        nc.vector.memset(biasT[:], -scale)
        nc.vector.memset(ct_aug[:].rearrange("p (t c) -> p t c", c=4)[:, :, 3:4], 1.0)

        # ---------------- load inputs ----------------
        nc.sync.dma_start(
            ft_all[:].rearrange("p (t d) -> p t d", d=D),
            ft_d.rearrange("(t p) d -> p t d", p=P))
        nc.sync.dma_start(
            fs_all[:].rearrange("p (t d) -> p t d", d=D),
            fs_d.rearrange("(t p) d -> p t d", p=P))
        nc.sync.dma_start(
            ct_all[:].rearrange("p (t c) -> p t c", c=3),
            ct_d.rearrange("(t p) c -> p t c", p=P))
        nc.sync.dma_start(
            cs_all[:].rearrange("p (t c) -> p t c", c=3),
            cs_d.rearrange("(t p) c -> p t c", p=P))

        ftv = ft_all[:].rearrange("p (t d) -> p t d", d=D)
        fsv = fs_all[:].rearrange("p (t d) -> p t d", d=D)
        ctv = ct_all[:].rearrange("p (t c) -> p t c", c=3)
        csv = cs_all[:].rearrange("p (t c) -> p t c", c=3)

        # ---------------- feature normalization + bf16 split ----------------
        def norm_split(flat, view, ntiles, s2, hi_all, lo_all):
            for t in range(ntiles):
                sq = scr.tile([P, D], F32, tag="sq")
                nc.scalar.activation(sq[:], view[:, t, :], AF.Square,
                                     accum_out=s2[:, t:t + 1])
            rn = scr.tile([P, ntiles], F32, tag="rn")
            nc.scalar.sqrt(rn[:], s2[:])
            nc.vector.reciprocal(rn[:], rn[:])
            for t in range(ntiles):
                nc.vector.tensor_scalar_mul(view[:, t, :], view[:, t, :],
                                            rn[:, t:t + 1])
            nc.vector.tensor_copy(hi_all[:], flat[:])
            if SPLIT_LO_MIXED:
                nc.vector.tensor_tensor(lo_all[:], flat[:], hi_all[:],
                                        op=ALU.subtract)
            else:
                hb = scr.tile(list(hi_all.shape), F32, tag="hb")
                nc.vector.tensor_copy(hb[:], hi_all[:])
                nc.vector.tensor_tensor(lo_all[:], flat[:], hb[:],
                                        op=ALU.subtract)

        norm_split(ft_all, ftv, MT, s2t, fhi_t, flo_t)
        norm_split(fs_all, fsv, NT, s2s, fhi_s, flo_s)

        fhiv_t = fhi_t[:].rearrange("p (t d) -> p t d", d=D)
        flov_t = flo_t[:].rearrange("p (t d) -> p t d", d=D)
        fhiv_s = fhi_s[:].rearrange("p (t d) -> p t d", d=D)
        flov_s = flo_s[:].rearrange("p (t d) -> p t d", d=D)

        # transposes: target features -> ftT  (stacked hi/lo)
        for t in range(MT):
            pt = psS.tile([D, P], BF16, tag="tp")
            nc.tensor.matmul(pt[:], fhiv_t[:, t, :], identb[:], is_transpose=True)
            nc.vector.tensor_copy(ftT[0:D, t * P:(t + 1) * P], pt[:])
            pt2 = psS.tile([D, P], BF16, tag="tp")
            nc.tensor.matmul(pt2[:], flov_t[:, t, :], identb[:], is_transpose=True)
            nc.vector.tensor_copy(ftT[D:2 * D, t * P:(t + 1) * P], pt2[:])
        # source features -> rhsA (hi, duplicated) and rhsB (lo)
        for t in range(NT):
            pt = psS.tile([D, P], BF16, tag="tp")
            nc.tensor.matmul(pt[:], fhiv_s[:, t, :], identb[:], is_transpose=True)
            nc.vector.tensor_copy(rhsA[0:D, t * P:(t + 1) * P], pt[:])
            nc.vector.tensor_copy(rhsA[D:2 * D, t * P:(t + 1) * P], pt[:])
            pt2 = psS.tile([D, P], BF16, tag="tp")
            nc.tensor.matmul(pt2[:], flov_s[:, t, :], identb[:], is_transpose=True)
            nc.vector.tensor_copy(rhsB[0:D, t * P:(t + 1) * P], pt2[:])

        # ---------------- coordinates ----------------
        for t in range(MT):
            ctc4 = scr.tile([P, 4], F32, tag="ctc")  # [-2*(ct-20) | 1]
            nc.vector.tensor_scalar_add(ctc4[:, 0:3], ctv[:, t, :], -CENTER)
            sq = scr.tile([P, 3], F32, tag="sqc")
            nc.scalar.activation(sq[:], ctc4[:, 0:3], AF.Square,
                                 accum_out=ct2c[:, t:t + 1])
            nc.vector.tensor_scalar_mul(ctc4[:, 0:3], ctc4[:, 0:3], -2.0)
            nc.vector.memset(ctc4[:, 3:4], 1.0)
            pt = psS.tile([4, P], F32, tag="tp")
            nc.tensor.matmul(pt[:], ctc4[:], ident[:], is_transpose=True)
            nc.vector.tensor_copy(ctT[:, t * P:(t + 1) * P], pt[:])
            nc.vector.tensor_copy(
                ct_aug[:].rearrange("p (t c) -> p t c", c=4)[:, t, 0:3],
                ctv[:, t, :])
        nc.vector.tensor_scalar(thr[:], ct2c[:], -1.0, RADIUS_SQ,
                                op0=ALU.mult, op1=ALU.add)

        for t in range(NT):
            csc4 = scr.tile([P, 4], F32, tag="csc")  # [(cs-20) | |cs-20|^2]
            nc.vector.tensor_scalar_add(csc4[:, 0:3], csv[:, t, :], -CENTER)
            sq = scr.tile([P, 3], F32, tag="sqc")
            nc.scalar.activation(sq[:], csc4[:, 0:3], AF.Square,
                                 accum_out=csc4[:, 3:4])
            pt = psS.tile([4, P], F32, tag="tp")
            nc.tensor.matmul(pt[:], csc4[:], ident[:], is_transpose=True)
            nc.vector.tensor_copy(rhsC[:, t * P:(t + 1) * P], pt[:])

        # ---------------- main loop ----------------
        for j in range(NCHUNK):
            cols = slice(j * CHUNK, (j + 1) * CHUNK)
            aggp = psG.tile([4, CHUNK], F32, tag="agg")
            for mt in range(MT):
                msl = slice(mt * P, (mt + 1) * P)
                sp = psA.tile([P, CHUNK], F32, tag="sp")
                nc.tensor.matmul(sp[:], ftT[:, msl], rhsA[:, cols],
                                 start=True, stop=False)
                nc.tensor.matmul(sp[:], ftT[0:D, msl], rhsB[:, cols],
                                 start=False, stop=True)
                rp = psB.tile([P, CHUNK], F32, tag="rp")
                nc.tensor.matmul(rp[:], ctT[:, msl], rhsC[:, cols],
                                 start=True, stop=True)
                e = sbE.tile([P, CHUNK], F32, tag="e")
                nc.scalar.activation(e[:], sp[:], AF.Exp, bias=biasT[:],
                                     scale=scale)
                k = sbK.tile([P, CHUNK], AGG_DT, tag="k")
                nc.vector.scalar_tensor_tensor(k[:], in0=rp[:],
                                               scalar=thr[:, mt:mt + 1],
                                               in1=e[:], op0=ALU.is_lt,
                                               op1=ALU.mult)
                nc.tensor.matmul(aggp[:], ct_aug[:, 4 * mt:4 * mt + 4], k[:],
                                 start=(mt == 0), stop=(mt == MT - 1))
            agg_sb = fin.tile([4, CHUNK], F32, tag="aggsb")
            nc.vector.tensor_copy(agg_sb[:], aggp[:])
            for tl in range(CHUNK // P):
                nt = j * (CHUNK // P) + tl
                tp = psS.tile([P, 4], F32, tag="tp")
                nc.tensor.matmul(tp[:], agg_sb[:, tl * P:(tl + 1) * P],
                                 ident[0:4, 0:4], is_transpose=True)
                tsb = fin.tile([P, 4], F32, tag="tsb")
                nc.vector.tensor_copy(tsb[:], tp[:])
                rec = fin.tile([P, 1], F32, tag="rec")
                nc.vector.tensor_scalar_add(rec[:], tsb[:, 3:4], 1e-8)
                nc.vector.reciprocal(rec[:], rec[:])
                res = fin.tile([P, 3], F32, tag="res")
                nc.vector.scalar_tensor_tensor(res[:], in0=tsb[:, 0:3],
                                               scalar=rec[:], in1=csv[:, nt, :],
                                               op0=ALU.mult, op1=ALU.subtract)
                nc.sync.dma_start(out_d[nt * P:(nt + 1) * P, :], res[:])

    nc.compile()
    return nc


_CACHE = {}


def kernel(feat_s, feat_t, coor_s, coor_t, epsilon):
    feat_s = np.ascontiguousarray(feat_s, dtype=np.float32)
    feat_t = np.ascontiguousarray(feat_t, dtype=np.float32)
    coor_s = np.ascontiguousarray(coor_s, dtype=np.float32)
    coor_t = np.ascontiguousarray(coor_t, dtype=np.float32)
    tau = float(np.exp(np.float32(epsilon)) + np.float32(TAU_OFFSET))

    key = round(tau, 12)
    if key not in _CACHE:
        _CACHE[key] = build_kernel(tau)
    nc = _CACHE[key]

    in_maps = []
    for c in range(N_CORES):
        b = c // CORES_PER_BATCH
        r = c % CORES_PER_BATCH
        sl = slice(r * NS, (r + 1) * NS)
        in_maps.append({
            "fs": np.ascontiguousarray(feat_s[b, sl]),
            "ft": feat_t[b],
            "cs": np.ascontiguousarray(coor_s[b, sl]),
            "ct": coor_t[b],
        })
    res = run_bass_kernel_spmd(nc, in_maps, core_ids=list(range(N_CORES)))
    out = np.empty((B, N, 3), dtype=np.float32)
    for c in range(N_CORES):
        b = c // CORES_PER_BATCH
        r = c % CORES_PER_BATCH
        out[b, r * NS:(r + 1) * NS] = res.results[c]["out"]
    return out
